# revision 4
# baseline (speedup 1.0000x reference)
"""Trainium2 Bass kernel: InterpretableMultiHeadAttention.

B=4, L=1024, D=1024, H=16, DK=64. Returns (output [B,L,D], attn [B,L,H,L]).

Sharding: data-parallel over (batch, query-half) -> 8 cores, no collectives.
Core i handles batch i//2, query rows (i%2)*512 ... +512 against all 1024 keys.

Per-core pipeline (all matmul operands bf16, accumulation fp32):
  A) q^T = (Wq^T x query^T)   [hdk, lq]   (+bias, *1/sqrt(dk), cast bf16)
     k^T = (Wk^T x key^T)     [hdk, lk]   (+bias)
     v    = value x wv + bv   [lk, dk]
  B) per head h:
     scores[lq,lk] = q_h^T.T @ k_h^T  -> ACT exp with accum_out => P_nat + denom
     attn = P_nat * (1/denom)  (DVE, per-partition scalar)  -> DMA out
     scores^T[lk,lq] = k_h^T.T @ q_h^T -> ACT exp => P^T (bf16)
     heads_h[lq,dk] = sum_lk P^T.T @ v  (PE, e-form) ; * (1/denom) into hm_all
  C) hm = sum_h hm_all (wo pre-divided by 16 on host), PE-transpose,
     out = hm^T.T @ wo + bo -> DMA out.

Host does the sharding, transposes and bf16 casts; device does all FLOPs.
"""

import sys

import numpy as np

if "/opt/trn_rl_repo" not in sys.path:
    sys.path.insert(0, "/opt/trn_rl_repo")

import ml_dtypes

B, L, D, H = 4, 1024, 1024, 16
DK = D // H  # 64
LQ = 512  # query rows per core
NCORES = 8

BF16 = ml_dtypes.bfloat16

_PROGRAM_CACHE = {}


def _build_program():
    import concourse.bass as bass
    import concourse.tile as tile
    from concourse import bacc, mybir
    from concourse.masks import make_identity

    f32 = mybir.dt.float32
    bf16 = mybir.dt.bfloat16
    ts = bass.ts

    nc = bacc.Bacc(None, target_bir_lowering=False)

    # ---- DRAM parameters (inputs) ----
    qt_d = nc.declare_dram_parameter("qt", [D, LQ], bf16, isOutput=False)
    kt_d = nc.declare_dram_parameter("kt", [D, L], bf16, isOutput=False)
    vt_d = nc.declare_dram_parameter("vt", [D, L], bf16, isOutput=False)
    wq_d = nc.declare_dram_parameter("wqr", [D, H * DK], bf16, isOutput=False)
    wk_d = nc.declare_dram_parameter("wkr", [D, H * DK], bf16, isOutput=False)
    wv_d = nc.declare_dram_parameter("wv", [D, DK], bf16, isOutput=False)
    wo_d = nc.declare_dram_parameter("wor", [DK, D], bf16, isOutput=False)
    bq_d = nc.declare_dram_parameter("bqc", [128, 8], f32, isOutput=False)
    bk_d = nc.declare_dram_parameter("bkc", [128, 8], f32, isOutput=False)
    bv_d = nc.declare_dram_parameter("bvr", [1, DK], bf16, isOutput=False)
    ones_d = nc.declare_dram_parameter("onesr", [1, 128], bf16, isOutput=False)
    bo_d = nc.declare_dram_parameter("bob", [128, D], f32, isOutput=False)

    # ---- DRAM outputs ----
    attn_d = nc.declare_dram_parameter("attn_p", [LQ, H, L], f32, isOutput=True)
    out_d = nc.declare_dram_parameter("out_p", [LQ, D], f32, isOutput=True)

    SCALE = 1.0 / 8.0  # 1/sqrt(DK)

    with tile.TileContext(nc) as tc:
        with (
            tc.tile_pool(name="const", bufs=1) as cpool,
            tc.tile_pool(name="proj", bufs=1) as ppool,
            tc.tile_pool(name="pnat", bufs=3) as pnat_pool,
            tc.tile_pool(name="attn", bufs=3) as attn_pool,
            tc.tile_pool(name="ptr", bufs=8) as pt_pool,
            tc.tile_pool(name="sden", bufs=8) as den_pool,
            tc.tile_pool(name="outs", bufs=2) as out_pool,
            tc.tile_pool(name="psA", bufs=3, space=bass.MemorySpace.PSUM) as psA,
            tc.tile_pool(name="psB", bufs=2, space=bass.MemorySpace.PSUM) as psB,
        ):
            # ---------- load constants / inputs ----------
            wq_sb = [cpool.tile([128, H * DK], bf16, name=f"wq{t}", tag=f"wq{t}") for t in range(8)]
            wk_sb = [cpool.tile([128, H * DK], bf16, name=f"wk{t}", tag=f"wk{t}") for t in range(8)]
            wv_sb = [cpool.tile([128, DK], bf16, name=f"wv{t}", tag=f"wv{t}") for t in range(8)]
            wo_sb = cpool.tile([DK, D], bf16, tag="wo")
            bq_sb = cpool.tile([128, 8], f32, tag="bq")
            bk_sb = cpool.tile([128, 8], f32, tag="bk")
            bv_sb = cpool.tile([1, DK], bf16, tag="bv")
            ones_sb = cpool.tile([1, 128], bf16, tag="ones")
            bo_sb = cpool.tile([128, D], f32, tag="bo")
            ident = cpool.tile([128, 128], f32, tag="ident")

            qt_sb = [cpool.tile([128, LQ], bf16, name=f"qt{t}", tag=f"qt{t}") for t in range(8)]
            kt_sb = [cpool.tile([128, L], bf16, name=f"kt{t}", tag=f"kt{t}") for t in range(8)]
            vt_sb = [cpool.tile([128, L], bf16, name=f"vt{t}", tag=f"vt{t}") for t in range(8)]

            for t in range(8):
                nc.sync.dma_start(wq_sb[t][:], wq_d[ts(t, 128), :])
                nc.sync.dma_start(wk_sb[t][:], wk_d[ts(t, 128), :])
                nc.sync.dma_start(wv_sb[t][:], wv_d[ts(t, 128), :])
                nc.sync.dma_start(qt_sb[t][:], qt_d[ts(t, 128), :])
                nc.sync.dma_start(kt_sb[t][:], kt_d[ts(t, 128), :])
                nc.sync.dma_start(vt_sb[t][:], vt_d[ts(t, 128), :])
            nc.sync.dma_start(wo_sb[:], wo_d[:])
            nc.sync.dma_start(bq_sb[:], bq_d[:])
            nc.sync.dma_start(bk_sb[:], bk_d[:])
            nc.sync.dma_start(bv_sb[:], bv_d[:])
            nc.sync.dma_start(ones_sb[:], ones_d[:])
            nc.sync.dma_start(bo_sb[:], bo_d[:])
            make_identity(nc, ident[:])

            # ---------- phase A: projections ----------
            q_sb = [ppool.tile([128, LQ], bf16, name=f"q{j}", tag=f"q{j}") for j in range(8)]
            k_sb = [ppool.tile([128, L], bf16, name=f"k{j}", tag=f"k{j}") for j in range(8)]
            v_sb = [ppool.tile([128, DK], bf16, name=f"v{i}", tag=f"v{i}") for i in range(8)]

            # q^T[hdk, lq]: accumulate over d tiles; +bq, *SCALE, cast bf16
            for j in range(8):
                ps = psA.tile([128, 1024], f32, tag="psA")
                for t in range(8):
                    nc.tensor.matmul(
                        ps[:, 0:LQ],
                        wq_sb[t][:, ts(j, 128)],
                        qt_sb[t][:],
                        start=(t == 0),
                        stop=(t == 7),
                    )
                nc.vector.tensor_scalar(
                    q_sb[j][:],
                    ps[:, 0:LQ],
                    bq_sb[:, j : j + 1],
                    SCALE,
                    op0=mybir.AluOpType.add,
                    op1=mybir.AluOpType.mult,
                )

            # k^T[hdk, lk]: +bk, cast bf16
            for j in range(8):
                for ck in range(2):
                    ps = psA.tile([128, 1024], f32, tag="psA")
                    for t in range(8):
                        nc.tensor.matmul(
                            ps[:, 0:512],
                            wk_sb[t][:, ts(j, 128)],
                            kt_sb[t][:, ts(ck, 512)],
                            start=(t == 0),
                            stop=(t == 7),
                        )
                    nc.vector.tensor_scalar(
                        k_sb[j][:, ts(ck, 512)],
                        ps[:, 0:512],
                        bk_sb[:, j : j + 1],
                        None,
                        op0=mybir.AluOpType.add,
                    )

            # v[lk, dk] = value @ wv + bv  (bias via K=1 matmul broadcast)
            for i in range(8):
                ps = psB.tile([128, 512], f32, tag="psB")
                for t in range(8):
                    nc.tensor.matmul(
                        ps[:, 0:DK],
                        vt_sb[t][:, ts(i, 128)],
                        wv_sb[t][:],
                        start=(t == 0),
                        stop=False,
                    )
                nc.tensor.matmul(
                    ps[:, 0:DK], ones_sb[:], bv_sb[:], start=False, stop=True
                )
                nc.vector.tensor_copy(v_sb[i][:], ps[:, 0:DK])

            # ---------- phase B: per-head attention ----------
            hm_all = [ppool.tile([128, DK, H], f32, name=f"hm{t}", tag=f"hm{t}") for t in range(4)]

            for h in range(H):
                jq = h // 2
                pq = (h % 2) * 64

                # B2: scores^T [lk, lq] -> exp -> P^T (bf16), 2 lk-tiles per group
                pts = []
                for g in range(4):
                    ps = psA.tile([128, 1024], f32, tag="psA")
                    for sub in range(2):
                        lk = 2 * g + sub
                        nc.tensor.matmul(
                            ps[:, ts(sub, 512)],
                            k_sb[jq][pq : pq + 64, ts(lk, 128)],
                            q_sb[jq][pq : pq + 64, :],
                            start=True,
                            stop=True,
                        )
                    pt = pt_pool.tile([128, 1024], bf16, tag="pt")
                    nc.scalar.activation(
                        pt[:], ps[:], mybir.ActivationFunctionType.Exp
                    )
                    pts.append(pt)

                for t in range(4):
                    # B1: scores [lq, lk] -> exp+accum -> P_nat, denom
                    ps = psA.tile([128, 1024], f32, tag="psA")
                    for ck in range(2):
                        nc.tensor.matmul(
                            ps[:, ts(ck, 512)],
                            q_sb[jq][pq : pq + 64, ts(t, 128)],
                            k_sb[jq][pq : pq + 64, ts(ck, 512)],
                            start=True,
                            stop=True,
                        )
                    pn = pnat_pool.tile([128, 1024], f32, tag="pn")
                    den = den_pool.tile([128, 2], f32, tag="den")
                    nc.scalar.activation(
                        pn[:],
                        ps[:],
                        mybir.ActivationFunctionType.Exp,
                        accum_out=den[:, 0:1],
                    )
                    nc.vector.reciprocal(den[:, 1:2], den[:, 0:1])

                    # attn = P_nat / denom -> DMA
                    at = attn_pool.tile([128, 1024], f32, tag="at")
                    nc.vector.tensor_scalar_mul(at[:], pn[:], den[:, 1:2])
                    nc.sync.dma_start(attn_d[ts(t, 128), h, :], at[:])

                    # B3: heads_h[lq, dk] = sum_lk P^T.T @ v ; * recip into hm_all
                    ph = psB.tile([128, 512], f32, tag="psB")
                    for lk in range(8):
                        g, sub = lk // 2, lk % 2
                        nc.tensor.matmul(
                            ph[:, 0:DK],
                            pts[g][:, sub * 512 + t * 128 : sub * 512 + (t + 1) * 128],
                            v_sb[lk][:],
                            start=(lk == 0),
                            stop=(lk == 7),
                        )
                    nc.vector.tensor_scalar_mul(
                        hm_all[t][:, :, h], ph[:, 0:DK], den[:, 1:2]
                    )

            # ---------- phase C: head mean + output projection ----------
            hmT = ppool.tile([DK, LQ], bf16, tag="hmT")
            for t in range(4):
                hr = out_pool.tile([128, DK], f32, tag="hr")
                nc.vector.tensor_reduce(
                    hr[:],
                    hm_all[t][:],
                    mybir.AxisListType.X,
                    mybir.AluOpType.add,
                )
                pt_ps = psB.tile([128, 512], f32, tag="psB")
                nc.tensor.transpose(pt_ps[0:DK, 0:128], hr[:], ident[:])
                nc.vector.tensor_copy(hmT[:, ts(t, 128)], pt_ps[0:DK, 0:128])

            for t in range(4):
                ot = out_pool.tile([128, D], f32, tag="ot")
                for ck in range(2):
                    po = psB.tile([128, 512], f32, tag="psB")
                    nc.tensor.matmul(
                        po[:, 0:512],
                        hmT[:, ts(t, 128)],
                        wo_sb[:, ts(ck, 512)],
                        start=True,
                        stop=True,
                    )
                    nc.vector.tensor_add(
                        ot[:, ts(ck, 512)], po[:, 0:512], bo_sb[:, ts(ck, 512)]
                    )
                nc.sync.dma_start(out_d[ts(t, 128), :], ot[:])

    nc.compile()
    return nc


def _prep_inputs(query, key_in, value, wq, bq, wk, bk, wv, bv, wo, bo):
    """Host-side shard + transpose + cast. Returns in_maps for 8 cores."""
    wq_r = np.ascontiguousarray(
        wq.transpose(1, 0, 2).reshape(D, H * DK), dtype=np.float32
    ).astype(BF16)
    wk_r = np.ascontiguousarray(
        wk.transpose(1, 0, 2).reshape(D, H * DK), dtype=np.float32
    ).astype(BF16)
    wv_b = np.ascontiguousarray(wv, dtype=np.float32).astype(BF16)
    wo_r = (np.ascontiguousarray(wo, dtype=np.float32) / np.float32(H)).astype(BF16)
    bq_c = np.ascontiguousarray(
        bq.reshape(H * DK).reshape(8, 128).T, dtype=np.float32
    )
    bk_c = np.ascontiguousarray(
        bk.reshape(H * DK).reshape(8, 128).T, dtype=np.float32
    )
    bv_r = bv.reshape(1, DK).astype(BF16)
    ones_r = np.ones((1, 128), dtype=BF16)
    bo_b = np.broadcast_to(bo.reshape(1, D), (128, D)).astype(np.float32).copy()

    shared = {
        "wqr": wq_r,
        "wkr": wk_r,
        "wv": wv_b,
        "wor": wo_r,
        "bqc": bq_c,
        "bkc": bk_c,
        "bvr": bv_r,
        "onesr": ones_r,
        "bob": bo_b,
    }

    in_maps = []
    for core in range(NCORES):
        b, half = core // 2, core % 2
        qs = np.ascontiguousarray(
            query[b, half * LQ : (half + 1) * LQ, :].T, dtype=np.float32
        ).astype(BF16)
        ks = np.ascontiguousarray(key_in[b].T, dtype=np.float32).astype(BF16)
        vs = np.ascontiguousarray(value[b].T, dtype=np.float32).astype(BF16)
        in_maps.append({"qt": qs, "kt": ks, "vt": vs, **shared})
    return in_maps


def _run(inputs, trace=False):
    from concourse.bass_utils import run_bass_kernel_spmd

    if "nc" not in _PROGRAM_CACHE:
        _PROGRAM_CACHE["nc"] = _build_program()
    nc = _PROGRAM_CACHE["nc"]

    in_maps = _prep_inputs(**inputs)
    res = run_bass_kernel_spmd(
        nc, in_maps, list(range(NCORES)), trace=trace
    )

    output = np.empty((B, L, D), dtype=np.float32)
    attn = np.empty((B, L, H, L), dtype=np.float32)
    for core in range(NCORES):
        b, half = core // 2, core % 2
        r = res.results[core]
        output[b, half * LQ : (half + 1) * LQ, :] = r["out_p"]
        attn[b, half * LQ : (half + 1) * LQ, :, :] = r["attn_p"]
    return (output, attn), res


def kernel(query, key_in, value, wq, bq, wk, bk, wv, bv, wo, bo):
    (output, attn), _ = _run(
        dict(
            query=np.asarray(query),
            key_in=np.asarray(key_in),
            value=np.asarray(value),
            wq=np.asarray(wq),
            bq=np.asarray(bq),
            wk=np.asarray(wk),
            bk=np.asarray(bk),
            wv=np.asarray(wv),
            bv=np.asarray(bv),
            wo=np.asarray(wo),
            bo=np.asarray(bo),
        )
    )
    return output, attn


# revision 12
# speedup vs baseline: 439.8248x; 439.8248x over previous
"""Trainium2 Bass kernel: InterpretableMultiHeadAttention.

B=4, L=1024, D=1024, H=16, DK=64. Returns (output [B,L,D], attn [B,L,H,L]).

Sharding: data-parallel over (batch, query-half) -> 8 cores, no collectives.
Core i handles batch i//2, query rows (i%2)*512 ... +512 against all 1024 keys.

Per-core pipeline (all matmul operands bf16, accumulation fp32):
  A) q^T = (Wq^T x query^T)   [hdk, lq]   (+bias, *1/sqrt(dk), cast bf16)
     k^T = (Wk^T x key^T)     [hdk, lk]   (+bias)
     v    = value x wv + bv   [lk, dk]
  B) per head h:
     scores[lq,lk] = q_h^T.T @ k_h^T  -> ACT exp with accum_out => P_nat + denom
     attn = P_nat * (1/denom)  (DVE, per-partition scalar)  -> DMA out
     scores^T[lk,lq] = k_h^T.T @ q_h^T -> ACT exp => P^T (bf16)
     heads_h[lq,dk] = sum_lk P^T.T @ v  (PE, e-form) ; * (1/denom) into hm_all
  C) hm = sum_h hm_all (wo pre-divided by 16 on host), PE-transpose,
     out = hm^T.T @ wo + bo -> DMA out.

Host does the sharding, transposes and bf16 casts; device does all FLOPs.
"""

import sys

import numpy as np

if "/opt/trn_rl_repo" not in sys.path:
    sys.path.insert(0, "/opt/trn_rl_repo")

import ml_dtypes

B, L, D, H = 4, 1024, 1024, 16
DK = D // H  # 64
LQ = 512  # query rows per core
NCORES = 8

BF16 = ml_dtypes.bfloat16

_PROGRAM_CACHE = {}


def _build_program():
    import concourse.bass as bass
    import concourse.tile as tile
    from concourse import bacc, mybir
    from concourse.masks import make_identity

    f32 = mybir.dt.float32
    bf16 = mybir.dt.bfloat16
    ts = bass.ts

    nc = bacc.Bacc(None, target_bir_lowering=False)

    # ---- DRAM parameters (inputs) ----
    qt_d = nc.declare_dram_parameter("qt", [D, LQ], bf16, isOutput=False)
    kt_d = nc.declare_dram_parameter("kt", [D, L], bf16, isOutput=False)
    vt_d = nc.declare_dram_parameter("vt", [D, L], bf16, isOutput=False)
    wq_d = nc.declare_dram_parameter("wqr", [D, H * DK], bf16, isOutput=False)
    wk_d = nc.declare_dram_parameter("wkr", [D, H * DK], bf16, isOutput=False)
    wv_d = nc.declare_dram_parameter("wv", [D, DK], bf16, isOutput=False)
    wo_d = nc.declare_dram_parameter("wor", [DK, D], bf16, isOutput=False)
    bq_d = nc.declare_dram_parameter("bqc", [128, 8], f32, isOutput=False)
    bk_d = nc.declare_dram_parameter("bkc", [128, 8], f32, isOutput=False)
    bv_d = nc.declare_dram_parameter("bvr", [1, DK], bf16, isOutput=False)
    ones_d = nc.declare_dram_parameter("onesr", [1, 128], bf16, isOutput=False)
    bo_d = nc.declare_dram_parameter("bob", [128, D], f32, isOutput=False)

    # ---- DRAM outputs ----
    attn_d = nc.declare_dram_parameter("attn_p", [LQ, H, L], f32, isOutput=True)
    out_d = nc.declare_dram_parameter("out_p", [LQ, D], f32, isOutput=True)

    SCALE = 1.0 / 8.0  # 1/sqrt(DK)

    with tile.TileContext(nc) as tc:
        with (
            tc.tile_pool(name="const", bufs=1) as cpool,
            tc.tile_pool(name="proj", bufs=1) as ppool,
            tc.tile_pool(name="pnat", bufs=3) as pnat_pool,
            tc.tile_pool(name="attn", bufs=3) as attn_pool,
            tc.tile_pool(name="ptr", bufs=12) as pt_pool,
            tc.tile_pool(name="sden", bufs=8) as den_pool,
            tc.tile_pool(name="outs", bufs=2) as out_pool,
            tc.tile_pool(name="psA", bufs=3, space=bass.MemorySpace.PSUM) as psA,
            tc.tile_pool(name="psB", bufs=2, space=bass.MemorySpace.PSUM) as psB,
        ):
            # ---------- load constants / inputs ----------
            wq_sb = [cpool.tile([128, H * DK], bf16, name=f"wq{t}", tag=f"wq{t}") for t in range(8)]
            wk_sb = [cpool.tile([128, H * DK], bf16, name=f"wk{t}", tag=f"wk{t}") for t in range(8)]
            wv_sb = [cpool.tile([128, DK], bf16, name=f"wv{t}", tag=f"wv{t}") for t in range(8)]
            wo_sb = cpool.tile([DK, D], bf16, tag="wo")
            bq_sb = cpool.tile([128, 8], f32, tag="bq")
            bk_sb = cpool.tile([128, 8], f32, tag="bk")
            bv_sb = cpool.tile([1, DK], bf16, tag="bv")
            ones_sb = cpool.tile([1, 128], bf16, tag="ones")
            bo_sb = cpool.tile([128, D], f32, tag="bo")
            ident = cpool.tile([128, 128], f32, tag="ident")

            qt_sb = [cpool.tile([128, LQ], bf16, name=f"qt{t}", tag=f"qt{t}") for t in range(8)]
            kt_sb = [cpool.tile([128, L], bf16, name=f"kt{t}", tag=f"kt{t}") for t in range(8)]
            vt_sb = [cpool.tile([128, L], bf16, name=f"vt{t}", tag=f"vt{t}") for t in range(8)]

            # loads gating the first q/k projection go first
            for t in range(8):
                nc.sync.dma_start(wq_sb[t][:], wq_d[ts(t, 128), :])
                nc.sync.dma_start(qt_sb[t][:], qt_d[ts(t, 128), :])
                nc.sync.dma_start(wk_sb[t][:], wk_d[ts(t, 128), :])
                nc.sync.dma_start(kt_sb[t][:], kt_d[ts(t, 128), :])
            for t in range(8):
                nc.sync.dma_start(wv_sb[t][:], wv_d[ts(t, 128), :])
                nc.sync.dma_start(vt_sb[t][:], vt_d[ts(t, 128), :])
            nc.sync.dma_start(wo_sb[:], wo_d[:])
            nc.sync.dma_start(bq_sb[:], bq_d[:])
            nc.sync.dma_start(bk_sb[:], bk_d[:])
            nc.sync.dma_start(bv_sb[:], bv_d[:])
            nc.sync.dma_start(ones_sb[:], ones_d[:])
            nc.sync.dma_start(bo_sb[:], bo_d[:])
            make_identity(nc, ident[:])

            # ---------- phase A: projections ----------
            q_sb = [ppool.tile([128, LQ], bf16, name=f"q{j}", tag=f"q{j}") for j in range(8)]
            k_sb = [ppool.tile([128, L], bf16, name=f"k{j}", tag=f"k{j}") for j in range(8)]
            v_sb = [ppool.tile([128, DK], bf16, name=f"v{i}", tag=f"v{i}") for i in range(8)]

            def project_v():
                # v[lk, dk] = value @ wv + bv (bias via K=1 matmul broadcast)
                for i in range(8):
                    ps = psB.tile([128, 512], f32, name=f"psv{i}", tag="psB")
                    for t in range(8):
                        nc.tensor.matmul(
                            ps[:, 0:DK],
                            vt_sb[t][:, ts(i, 128)],
                            wv_sb[t][:],
                            start=(t == 0),
                            stop=False,
                        )
                    nc.tensor.matmul(
                        ps[:, 0:DK], ones_sb[:], bv_sb[:], start=False, stop=True
                    )
                    nc.vector.tensor_copy(v_sb[i][:], ps[:, 0:DK])

            def project_qk(j):
                # q^T[hdk, lq]: accumulate over d tiles; +bq, *SCALE, bf16
                ps = psA.tile([128, 1024], f32, name=f"psq{j}", tag="psA")
                for t in range(8):
                    nc.tensor.matmul(
                        ps[:, 0:LQ],
                        wq_sb[t][:, ts(j, 128)],
                        qt_sb[t][:],
                        start=(t == 0),
                        stop=(t == 7),
                    )
                nc.vector.tensor_scalar(
                    q_sb[j][:],
                    ps[:, 0:LQ],
                    bq_sb[:, j : j + 1],
                    SCALE,
                    op0=mybir.AluOpType.add,
                    op1=mybir.AluOpType.mult,
                )
                # k^T[hdk, lk]: +bk, cast bf16
                for ck in range(2):
                    ps = psA.tile([128, 1024], f32, name=f"psk{j}{ck}", tag="psA")
                    for t in range(8):
                        nc.tensor.matmul(
                            ps[:, 0:512],
                            wk_sb[t][:, ts(j, 128)],
                            kt_sb[t][:, ts(ck, 512)],
                            start=(t == 0),
                            stop=(t == 7),
                        )
                    nc.vector.tensor_scalar(
                        k_sb[j][:, ts(ck, 512)],
                        ps[:, 0:512],
                        bk_sb[:, j : j + 1],
                        None,
                        op0=mybir.AluOpType.add,
                    )

            # ---------- phase B: per-head attention ----------
            hm_all = [ppool.tile([128, DK, H], f32, name=f"hm{t}", tag=f"hm{t}") for t in range(4)]

            # heads processed in pairs: head 2*hp at partitions 0-63, head
            # 2*hp+1 at 64-127 of the same q/k tiles -> interleaved K=64
            # matmuls run concurrently in separate PE row groups.
            project_qk(0)
            project_v()

            for hp in range(8):
                jq = hp

                # B2: scores^T [lk, lq] -> exp -> P^T (bf16), 2 lk-tiles/group
                pts2 = ([], [])
                for g in range(4):
                    psp = [
                        psA.tile([128, 1024], f32, name=f"psA_{hp}_{g}_{e}", tag="psA")
                        for e in range(2)
                    ]
                    for sub in range(2):
                        lk = 2 * g + sub
                        for e in range(2):
                            pq = e * 64
                            nc.tensor.matmul(
                                psp[e][:, ts(sub, 512)],
                                k_sb[jq][pq : pq + 64, ts(lk, 128)],
                                q_sb[jq][pq : pq + 64, :],
                                start=True,
                                stop=True,
                            )
                    for e in range(2):
                        pt = pt_pool.tile([128, 1024], bf16, name="pt", tag="pt")
                        nc.scalar.activation(
                            pt[:], psp[e][:], mybir.ActivationFunctionType.Exp
                        )
                        pts2[e].append(pt)

                if hp < 7:
                    project_qk(hp + 1)

                for t in range(4):
                    dens = []
                    pss = []
                    # B1 matmuls for both heads, interleaved by row group
                    for e in range(2):
                        ps = psA.tile(
                            [128, 1024], f32, name=f"psB1_{hp}_{t}_{e}", tag="psA"
                        )
                        pss.append(ps)
                    for ck in range(2):
                        for e in range(2):
                            pq = e * 64
                            nc.tensor.matmul(
                                pss[e][:, ts(ck, 512)],
                                q_sb[jq][pq : pq + 64, ts(t, 128)],
                                k_sb[jq][pq : pq + 64, ts(ck, 512)],
                                start=True,
                                stop=True,
                            )
                    for e in range(2):
                        h = 2 * hp + e
                        pn = pnat_pool.tile([128, 1024], f32, name="pn", tag="pn")
                        den = den_pool.tile([128, 2], f32, name="den", tag="den")
                        nc.scalar.activation(
                            pn[:],
                            pss[e][:],
                            mybir.ActivationFunctionType.Exp,
                            accum_out=den[:, 0:1],
                        )
                        nc.vector.reciprocal(den[:, 1:2], den[:, 0:1])
                        dens.append(den)

                        # attn = P_nat / denom -> DMA
                        at = attn_pool.tile([128, 1024], f32, name="at", tag="at")
                        nc.vector.tensor_scalar_mul(at[:], pn[:], den[:, 1:2])
                        nc.gpsimd.dma_start(attn_d[ts(t, 128), h, :], at[:])

                    # B3: heads[lq, dk] = sum_lk P^T.T @ v ; * recip into hm_all
                    for e in range(2):
                        h = 2 * hp + e
                        ph = psB.tile(
                            [128, 512], f32, name=f"ph_{hp}_{t}_{e}", tag="psB"
                        )
                        for lk in range(8):
                            g, sub = lk // 2, lk % 2
                            nc.tensor.matmul(
                                ph[:, 0:DK],
                                pts2[e][g][
                                    :, sub * 512 + t * 128 : sub * 512 + (t + 1) * 128
                                ],
                                v_sb[lk][:],
                                start=(lk == 0),
                                stop=(lk == 7),
                            )
                        nc.vector.tensor_scalar_mul(
                            hm_all[t][:, :, h], ph[:, 0:DK], dens[e][:, 1:2]
                        )

            # ---------- phase C: head mean + output projection ----------
            hmT = ppool.tile([DK, LQ], bf16, tag="hmT")
            for t in range(4):
                hr = out_pool.tile([128, DK], f32, tag="hr")
                nc.vector.tensor_reduce(
                    hr[:],
                    hm_all[t][:],
                    mybir.AxisListType.X,
                    mybir.AluOpType.add,
                )
                pt_ps = psB.tile([128, 512], f32, tag="psB")
                nc.tensor.transpose(pt_ps[0:DK, 0:128], hr[:], ident[:])
                nc.vector.tensor_copy(hmT[:, ts(t, 128)], pt_ps[0:DK, 0:128])

            for t in range(4):
                ot = out_pool.tile([128, D], f32, tag="ot")
                for ck in range(2):
                    po = psB.tile([128, 512], f32, tag="psB")
                    nc.tensor.matmul(
                        po[:, 0:512],
                        hmT[:, ts(t, 128)],
                        wo_sb[:, ts(ck, 512)],
                        start=True,
                        stop=True,
                    )
                    nc.vector.tensor_add(
                        ot[:, ts(ck, 512)], po[:, 0:512], bo_sb[:, ts(ck, 512)]
                    )
                nc.sync.dma_start(out_d[ts(t, 128), :], ot[:])

    nc.compile()
    return nc


def _prep_inputs(query, key_in, value, wq, bq, wk, bk, wv, bv, wo, bo):
    """Host-side shard + transpose + cast. Returns in_maps for 8 cores."""
    wq_r = np.ascontiguousarray(
        wq.transpose(1, 0, 2).reshape(D, H * DK), dtype=np.float32
    ).astype(BF16)
    wk_r = np.ascontiguousarray(
        wk.transpose(1, 0, 2).reshape(D, H * DK), dtype=np.float32
    ).astype(BF16)
    wv_b = np.ascontiguousarray(wv, dtype=np.float32).astype(BF16)
    wo_r = (np.ascontiguousarray(wo, dtype=np.float32) / np.float32(H)).astype(BF16)
    bq_c = np.ascontiguousarray(
        bq.reshape(H * DK).reshape(8, 128).T, dtype=np.float32
    )
    bk_c = np.ascontiguousarray(
        bk.reshape(H * DK).reshape(8, 128).T, dtype=np.float32
    )
    bv_r = bv.reshape(1, DK).astype(BF16)
    ones_r = np.ones((1, 128), dtype=BF16)
    bo_b = np.broadcast_to(bo.reshape(1, D), (128, D)).astype(np.float32).copy()

    shared = {
        "wqr": wq_r,
        "wkr": wk_r,
        "wv": wv_b,
        "wor": wo_r,
        "bqc": bq_c,
        "bkc": bk_c,
        "bvr": bv_r,
        "onesr": ones_r,
        "bob": bo_b,
    }

    in_maps = []
    for core in range(NCORES):
        b, half = core // 2, core % 2
        qs = np.ascontiguousarray(
            query[b, half * LQ : (half + 1) * LQ, :].T, dtype=np.float32
        ).astype(BF16)
        ks = np.ascontiguousarray(key_in[b].T, dtype=np.float32).astype(BF16)
        vs = np.ascontiguousarray(value[b].T, dtype=np.float32).astype(BF16)
        in_maps.append({"qt": qs, "kt": ks, "vt": vs, **shared})
    return in_maps


def _run(inputs, trace=False):
    from concourse.bass_utils import run_bass_kernel_spmd

    if "nc" not in _PROGRAM_CACHE:
        _PROGRAM_CACHE["nc"] = _build_program()
    nc = _PROGRAM_CACHE["nc"]

    in_maps = _prep_inputs(**inputs)
    res = run_bass_kernel_spmd(
        nc, in_maps, list(range(NCORES)), trace=trace
    )

    output = np.empty((B, L, D), dtype=np.float32)
    attn = np.empty((B, L, H, L), dtype=np.float32)
    for core in range(NCORES):
        b, half = core // 2, core % 2
        r = res.results[core]
        output[b, half * LQ : (half + 1) * LQ, :] = r["out_p"]
        attn[b, half * LQ : (half + 1) * LQ, :, :] = r["attn_p"]
    return (output, attn), res


def kernel(query, key_in, value, wq, bq, wk, bk, wv, bv, wo, bo):
    (output, attn), _ = _run(
        dict(
            query=np.asarray(query),
            key_in=np.asarray(key_in),
            value=np.asarray(value),
            wq=np.asarray(wq),
            bq=np.asarray(bq),
            wk=np.asarray(wk),
            bk=np.asarray(bk),
            wv=np.asarray(wv),
            bv=np.asarray(bv),
            wo=np.asarray(wo),
            bo=np.asarray(bo),
        )
    )
    return output, attn
